# revision 12
# baseline (speedup 1.0000x reference)
"""Trainium2 Bass kernel: CodeEncoder attention pooling.

Math (per (b,v) bag): gather 64 embeddings [C=64, D=256] from a 20000x256
table, score each code with score = W2 @ tanh(W1 @ e + b1) (+b2, dropped:
softmax is shift-invariant), masked softmax over the 64 codes (c < length,
else -1e9), output = sum_c attn[c] * e[c].

Key structure: the score of a code depends only on its vocab id, so each
core computes a score table over the vocab once (tiny MLP over the
transposed table); per-code scores are then a 2-byte indirect gather.

Sharding: data-parallel over batch B=64 -> 8 batches per core on 8 cores.
Per core: 400 bags, 25600 gathered rows.

Pipeline per core (f16 data, fp8 W1 layer, f32 accumulation):
  1. Embedding gather via indirect_dma_start (mainline SWDGE, ~0.4ns per
     descriptor vs ~8.4ns/idx for the Q7 dma_gather ucode): 25600 rows of
     512B from the table into emb [128p, 200, 256]; slot g*128+p holds
     code (bag, c) = divmod(g*128+p, 64). 4 calls of <=8192 descriptors.
  2. Score table: for v-chunks of the fp8 transposed table [128, 2, VPAD],
     one DoubleRow fp8 matmul per 512-slice does the full 256-deep W1
     contraction; tanh on ACT; w = W2rep @ h (scores replicated on all
     128 psum partitions) -> f16 chunk; partition 0's row is written to a
     scratch DRAM score table [1, VPAD].
  3. Per-code scores: one indirect_dma_start per bag-tile gathers 8192
     2-byte scores straight into bag-major scores_sb [128, NT, 64].
  4. Masked softmax along free axis; PE-transpose attn -> [64, 128]; build
     block-diagonal lhsT [128, 16, 32] per 32-row psum stripe with 2
     strided copies.
  5. Pooling: per bag pair one matmul lhsT=[128,32] slot block, rhs=emb
     block [128,256] -> psum rows; one DVE copy + one output DMA per tile.
"""

import sys

if "/opt/trn_rl_repo" not in sys.path:
    sys.path.insert(0, "/opt/trn_rl_repo")

from contextlib import ExitStack

import numpy as np

B, V, C = 64, 50, 64
NUM_CODE, D, H = 20000, 256, 128
NCORES = 8
BPC = B // NCORES          # batches per core
BAGS = BPC * V             # 400 bags per core
CODES = BAGS * C           # 25600 codes per core
NBLK = CODES // 128        # 200 column-blocks of gathered embeddings
GCH = 8                    # emb gather chunk in blocks (1024 descriptors;
                           # the SWDGE descriptor ring holds ~64-200/engine)
CSL = 8                    # score gather chunk in c columns (1024 descs)
TILE_BAGS = (128, 128, 128, 16)  # bags per softmax/pooling tile
NT = len(TILE_BAGS)
VPAD = 20480               # vocab padded for clean chunking
VCH = 2048                 # score-table build chunk
NCH = VPAD // VCH          # 10 chunks
NSL = 512                  # matmul N slice (f32 psum bank)

_cache = {}


def _build_program():
    import concourse.bass as bass
    import concourse.tile as tile
    from concourse import bacc, mybir

    f8 = mybir.dt.float8e4
    f16 = mybir.dt.float16
    f32 = mybir.dt.float32
    i32 = mybir.dt.int32

    nc = bacc.Bacc("TRN2", target_bir_lowering=False, debug=False,
                   num_devices=NCORES)

    table_d = nc.dram_tensor("table", [NUM_CODE, D], f16, kind="ExternalInput")
    tableT8_d = nc.dram_tensor("tableT8", [128, 2, VPAD], f8,
                               kind="ExternalInput")
    w1t8_d = nc.dram_tensor("w1t8", [128, 2, H], f8, kind="ExternalInput")
    w2rep_d = nc.dram_tensor("w2rep", [H, 128], f16, kind="ExternalInput")
    b1_d = nc.dram_tensor("b1", [H, 1], f32, kind="ExternalInput")
    oidx_d = nc.dram_tensor("oidx", [128, NBLK], i32, kind="ExternalInput")
    sidx_d = nc.dram_tensor("sidx", [128, NT, C], i32, kind="ExternalInput")
    lens_d = nc.dram_tensor("lens", [128, NT], f32, kind="ExternalInput")
    cvals_d = nc.dram_tensor("cvals", [128, C], f32, kind="ExternalInput")
    ident_d = nc.dram_tensor("ident", [128, 128], f16, kind="ExternalInput")
    scored_d = nc.dram_tensor("scored", [1, VPAD], f16, kind="Internal")
    out_d = nc.dram_tensor("out", [BAGS, D], f32, kind="ExternalOutput")

    with tile.TileContext(nc) as tc, ExitStack() as ctx:
        const = ctx.enter_context(tc.tile_pool(name="const", bufs=1))
        tabp = ctx.enter_context(tc.tile_pool(name="tabp", bufs=2))
        scp = ctx.enter_context(tc.tile_pool(name="scp", bufs=2))
        hp = ctx.enter_context(tc.tile_pool(name="hp", bufs=4))
        soft = ctx.enter_context(tc.tile_pool(name="soft", bufs=2))
        blkp = ctx.enter_context(tc.tile_pool(name="blkp", bufs=NT))
        outp = ctx.enter_context(tc.tile_pool(name="outp", bufs=2))
        ph_p = ctx.enter_context(tc.tile_pool(name="ph", bufs=2, space="PSUM"))
        pw_p = ctx.enter_context(tc.tile_pool(name="pw", bufs=2, space="PSUM"))
        ptr_p = ctx.enter_context(tc.tile_pool(name="ptr", bufs=2, space="PSUM"))
        ppool_p = ctx.enter_context(tc.tile_pool(name="ppool", bufs=2,
                                                 space="PSUM"))

        # --- constant / input uploads (HWDGE); gather offsets first ---
        oidx_sb = const.tile([128, NBLK], i32)
        nc.sync.dma_start(oidx_sb[:], oidx_d.ap())
        sidx_sb = const.tile([128, NT, C], i32)
        nc.sync.dma_start(sidx_sb[:], sidx_d.ap())
        w1t8_sb = const.tile([128, 2, H], f8)
        nc.sync.dma_start(w1t8_sb[:], w1t8_d.ap())
        w2rep_sb = const.tile([H, 128], f16)
        nc.sync.dma_start(w2rep_sb[:], w2rep_d.ap())
        b1_sb = const.tile([H, 1], f32)
        nc.sync.dma_start(b1_sb[:], b1_d.ap())
        lens_sb = const.tile([128, NT], f32)
        nc.sync.dma_start(lens_sb[:], lens_d.ap())
        cvals_sb = const.tile([128, C], f32)
        nc.sync.dma_start(cvals_sb[:], cvals_d.ap())
        ident_sb = const.tile([128, 128], f16)
        nc.sync.dma_start(ident_sb[:], ident_d.ap())

        # --- embedding gather (indirect, 512B rows) ---
        emb_sb = const.tile([128, NBLK, D], f16)
        for g0 in range(0, NBLK, GCH):
            g1 = min(g0 + GCH, NBLK)
            nc.gpsimd.indirect_dma_start(
                out=emb_sb[:, g0:g1, :],
                out_offset=None,
                in_=table_d.ap(),
                in_offset=bass.IndirectOffsetOnAxis(
                    ap=oidx_sb[:, g0:g1], axis=0),
            )

        # --- score table build; partition-0 rows stream to DRAM ---
        tableT8_ap = tableT8_d.ap()
        scored_ap = scored_d.ap()
        ncp = 0
        for ci in range(NCH):
            tab_t = tabp.tile([128, 2, VCH], f8)
            sl = slice(ci * VCH, (ci + 1) * VCH)
            nc.sync.dma_start(tab_t[:], tableT8_ap[:, :, sl])
            sc_chunk = scp.tile([128, VCH], f16)
            for ni in range(VCH // NSL):
                nsl = slice(ni * NSL, (ni + 1) * NSL)
                ph = ph_p.tile([128, NSL], f32)
                nc.tensor.matmul(ph[:], w1t8_sb[:], tab_t[:, :, nsl],
                                 start=True, stop=True,
                                 perf_mode=mybir.MatmulPerfMode.DoubleRow)
                h_sb = hp.tile([128, NSL], f16)
                nc.scalar.activation(h_sb[:], ph[:],
                                     mybir.ActivationFunctionType.Tanh,
                                     bias=b1_sb[:], scale=1.0)
                pw = pw_p.tile([128, NSL], f32)
                nc.tensor.matmul(pw[:], w2rep_sb[:], h_sb[:],
                                 start=True, stop=True)
                # every psum partition holds the same scores; cast to f16
                if ncp % 2 == 0:
                    nc.vector.tensor_copy(sc_chunk[:, nsl], pw[:])
                else:
                    nc.scalar.copy(sc_chunk[:, nsl], pw[:])
                ncp += 1
            nc.sync.dma_start(scored_ap[:, sl], sc_chunk[0:1, :])

        # --- per-code scores (indirect, 2B each) + masked softmax ---
        scores_sb = const.tile([128, NT, C], f16)
        for t in range(NT):
            for c0 in range(0, C, CSL):
                nc.gpsimd.indirect_dma_start(
                    out=scores_sb[:, t, c0:c0 + CSL],
                    out_offset=None,
                    in_=scored_ap,
                    in_offset=bass.IndirectOffsetOnAxis(
                        ap=sidx_sb[:, t, c0:c0 + CSL], axis=1),
                )
            m01 = soft.tile([128, C], f32, tag="m01")
            nc.vector.tensor_scalar(m01[:], cvals_sb[:], lens_sb[:, t:t + 1],
                                    None, mybir.AluOpType.is_lt)
            madd = soft.tile([128, C], f32, tag="madd")
            nc.vector.tensor_scalar(madd[:], m01[:], 1.0, 1e9,
                                    mybir.AluOpType.subtract,
                                    mybir.AluOpType.mult)
            s32 = soft.tile([128, C], f32, tag="s32")
            nc.vector.tensor_copy(s32[:], scores_sb[:, t, :])
            wm = soft.tile([128, C], f32, tag="wm")
            nc.vector.tensor_mul(wm[:], s32[:], m01[:])
            nc.vector.tensor_add(wm[:], wm[:], madd[:])
            nmx = soft.tile([128, 1], f32, tag="nmx")
            nc.vector.tensor_reduce(nmx[:], wm[:], mybir.AxisListType.X,
                                    mybir.AluOpType.max, negate=True)
            ex = soft.tile([128, C], f32, tag="ex")
            sm = soft.tile([128, 1], f32, tag="sm")
            nc.scalar.activation(ex[:], wm[:],
                                 mybir.ActivationFunctionType.Exp,
                                 bias=nmx[:], scale=1.0, accum_out=sm[:])
            rs = soft.tile([128, 1], f32, tag="rs")
            nc.vector.reciprocal(rs[:], sm[:])
            attn = soft.tile([128, C], f16, tag="attn")
            nc.vector.tensor_scalar(attn[:], ex[:], rs[:], None,
                                    mybir.AluOpType.mult)
            # transpose attn -> [c, bag]
            ptr = ptr_p.tile([C, 128], f16)
            nc.tensor.transpose(ptr[:], attn[:], ident_sb[:])
            attnT = soft.tile([C, 128], f16, tag="attnT")
            nc.vector.tensor_copy(attnT[:], ptr[:])
            # Pooling. PE output base partitions must be 32-aligned, so
            # pairs are grouped 16 per 32-row psum stripe: pair slot s of
            # group j uses lhsT [128, 32] with only columns 2s (rows 0:64 =
            # even bag's attn) and 2s+1 (rows 64:128 = odd bag) nonzero;
            # the 16 matmuls accumulate into psum[32j:32j+32].
            # The (s -> column 2s) structure is a diagonal, built with two
            # strided copies: flat offset s*32 + 2s = 34s.
            nb = TILE_BAGS[t]
            ppool = ppool_p.tile([128, D], f32)
            at_ap = attnT[:]
            for j in range((nb + 31) // 32):
                npair = min(16, nb // 2 - 16 * j)
                blockT = blkp.tile([128, 16, 32], f16)
                nc.vector.memset(blockT[:], 0)
                bt_ap = blockT[:]
                dst_even = bass.AP(bt_ap.tensor, bt_ap.offset,
                                   [[512, C], [34, npair], [1, 1]])
                dst_odd = bass.AP(bt_ap.tensor, bt_ap.offset + C * 512 + 1,
                                  [[512, C], [34, npair], [1, 1]])
                src_even = bass.AP(at_ap.tensor, at_ap.offset + 32 * j,
                                   [[128, C], [2, npair], [1, 1]])
                src_odd = bass.AP(at_ap.tensor, at_ap.offset + 32 * j + 1,
                                  [[128, C], [2, npair], [1, 1]])
                nc.vector.tensor_copy(dst_even, src_even)
                nc.vector.tensor_copy(dst_odd, src_odd)
                for s in range(npair):
                    nc.tensor.matmul(ppool[32 * j:32 * j + 32, :],
                                     blockT[:, s, :],
                                     emb_sb[:, 64 * t + 16 * j + s, :],
                                     start=(s == 0), stop=(s == npair - 1),
                                     tile_position=(0, 32 * j))
            out_sb = outp.tile([128, D], f32)
            nc.vector.tensor_copy(out_sb[0:nb, :], ppool[0:nb, :])
            nc.sync.dma_start(out_d.ap()[128 * t:128 * t + nb, :],
                              out_sb[0:nb, :])

    nc.compile()
    return nc


def _prep_shared(embed_table, W1, b1, W2):
    import ml_dtypes

    f8 = ml_dtypes.float8_e4m3
    tab16 = embed_table.astype(np.float16)                    # [20000, 256]
    tT = embed_table.T.astype(f8)                             # [256, 20000]
    tableT8 = np.zeros((128, 2, VPAD), dtype=f8)
    tableT8[:, 0, :NUM_CODE] = tT[0:128]
    tableT8[:, 1, :NUM_CODE] = tT[128:256]
    # w1t8[p, j, h] = W1[h, 128*j + p]  (DoubleRow k-tile layout)
    w1t8 = np.ascontiguousarray(
        W1.astype(f8).T.reshape(2, 128, H).transpose(1, 0, 2))
    w2rep = np.repeat(W2.astype(np.float16).reshape(H, 1), 128, axis=1)
    b1c = np.ascontiguousarray(b1.astype(np.float32).reshape(H, 1))
    cvals = np.broadcast_to(np.arange(C, dtype=np.float32), (128, C)).copy()
    ident = np.eye(128, dtype=np.float16)
    return dict(table=tab16, tableT8=tableT8, w1t8=w1t8, w2rep=w2rep, b1=b1c,
                cvals=cvals, ident=ident)


def build_in_maps(input_code, length_code, shared):
    in_maps = []
    for core in range(NCORES):
        bs = slice(core * BPC, (core + 1) * BPC)
        codes = input_code[bs].reshape(-1).astype(np.int32)     # [25600]
        # emb slot g*128+p holds code (bag,c) = slot -> oidx[p, g]
        oidx = np.ascontiguousarray(codes.reshape(NBLK, 128).T)  # [128, 200]
        # scores_sb[p, t, c] <- code of bag 128t+p, position c
        cp = np.zeros((NT * 128, C), dtype=np.int32)
        cp[:BAGS] = codes.reshape(BAGS, C)
        sidx = np.ascontiguousarray(
            cp.reshape(NT, 128, C).transpose(1, 0, 2))           # [128,NT,C]
        lens = np.full((128, NT), C, dtype=np.float32)
        lv = length_code[bs].reshape(-1).astype(np.float32)     # [400]
        for t in range(NT):
            lens[:TILE_BAGS[t], t] = lv[128 * t:128 * t + TILE_BAGS[t]]
        in_maps.append(dict(shared, oidx=oidx, sidx=sidx, lens=lens))
    return in_maps


def kernel(input_code, length_code, embed_table, W1, b1, W2, b2):
    from concourse.bass_utils import run_bass_kernel_spmd

    if "nc" not in _cache:
        _cache["nc"] = _build_program()
    nc = _cache["nc"]

    shared = _prep_shared(np.asarray(embed_table), np.asarray(W1),
                          np.asarray(b1), np.asarray(W2))
    input_code = np.asarray(input_code)
    length_code = np.asarray(length_code)

    in_maps = build_in_maps(input_code, length_code, shared)
    res = run_bass_kernel_spmd(nc, in_maps, core_ids=list(range(NCORES)))
    outs = [res.results[c]["out"].reshape(BPC, V, D) for c in range(NCORES)]
    return np.concatenate(outs, axis=0)
